# revision 14
# baseline (speedup 1.0000x reference)
"""Trainium2 Bass kernel for nn_AutoregressiveRoutingHead (v2).

Model (per batch row b):
    tok_in = [START, tgt[0..6]]                       # teacher forcing, START=5
    x_t    = emb[tok_in[t]]                           # (HID,)
    gi     = x_t @ W_ih.T + b_ih                      # (768,)
    gh     = h @ W_hh.T + b_hh                        # (768,)
    r = sigmoid(gi_r + gh_r); z = sigmoid(gi_z + gh_z)
    n = tanh(gi_n + r * gh_n)
    h' = n - z*(n - h)
    logits_t = h' @ W_out.T + b_out                   # (5,)

Strategy (pure data parallel over batch, 65536 -> 8 x 8192):
  * All per-token quantities are host-precomputed: the latent h0 arrives
    pre-transposed/pre-cast as f16 [128, KC, b], the token onehots for all 8
    steps (incl. START at t=0) arrive as f16 [8, L, b], and the 6x768 gi table
    (emb @ W_ih.T + b_ih + b_hh_rz) is packed so each 128-row gate chunk's
    slice sits at partition rows 32*m (4-way row-tiled K=8 matmuls).
  * Steps are uniform: rz gates = onehot-MM (start) + 2 W_hh MMs per chunk
    into two 2-bank PSUM tiles, one sigmoid per tile; n-part = 4 W_hh MMs into
    a 2-bank PSUM tile, then DVE computes r*gh_n IN PLACE in PSUM and the PE
    accumulates gi_n on top (has_written bits survive the DVE overwrite), so
    tanh reads the finished preactivation straight from PSUM.
  * Logits are deferred: h'_t for all 8 steps is kept in SBUF; at chunk end 16
    col-tiled MMs put steps 4b+g at PSUM partitions 32g of bank b, one ACT
    copy extracts them, GPSIMD-queue DMAs write them out (f16; host upcasts).
  * 4 chunks in flight; per step-round the emission is software-pipelined
    (second half of the update lagged by one chunk) so no engine queue ever
    head-of-line blocks on the recurrence chain.
"""

import numpy as np

import concourse.bass as bass
import concourse.mybir as mybir
import concourse.tile as tile
from concourse import bacc, bass_utils

F32 = mybir.dt.float32
F16 = mybir.dt.float16
F8 = mybir.dt.float8e4
DR = mybir.MatmulPerfMode.DoubleRow
AF = mybir.ActivationFunctionType
ALU = mybir.AluOpType

N_CORES = 8
B = 65536
L = 8
LATENT = 256
HID = 128
NTOK = 5
V = NTOK + 1  # vocab incl <start>
START = NTOK
G = 3 * LATENT  # 768 gate rows
KC = LATENT // 128  # 2 contraction chunks

B_CORE = B // N_CORES


def build_program(b_core=B_CORE, n_b=512, group=4, use_bhhn=False, dr=0,
                  eg=False):
    """Build + compile the per-core Bass program (SPMD: same program, 8 cores).

    dr=0: all-f16 matmuls; dr=1: fp8 DoubleRow for the rz-gate W_hh matmuls;
    dr=2: fp8 DoubleRow for rz and n-gate W_hh matmuls.
    """
    nc = bacc.Bacc("TRN2", target_bir_lowering=False, debug=False)
    n_chunks = b_core // n_b
    assert n_chunks * n_b == b_core

    # ---- DRAM I/O ----------------------------------------------------------
    latT = nc.dram_tensor("latT", [128, KC, b_core], F16, kind="ExternalInput").ap()
    ohd = nc.dram_tensor("ohd", [8, L, b_core], F16, kind="ExternalInput").ap()
    gi = nc.dram_tensor("gi", [128, 2, 128], F16, kind="ExternalInput").ap()
    whh = nc.dram_tensor("whh", [128, KC, G], F16, kind="ExternalInput").ap()
    whh8 = None
    if dr:
        whh8 = nc.dram_tensor("whh8", [128, KC, G], F8, kind="ExternalInput").ap()
    wout = nc.dram_tensor("wout", [128, KC, NTOK], F16, kind="ExternalInput").ap()
    bout = nc.dram_tensor("bout", [1, 128], F16, kind="ExternalInput").ap()
    bhhn = None
    if use_bhhn:
        bhhn = nc.dram_tensor("bhhn", [1, LATENT], F16, kind="ExternalInput").ap()
    out16 = nc.dram_tensor("out16", [L, NTOK, b_core], F16, kind="ExternalOutput").ap()

    with tile.TileContext(nc) as tc:
        with tc.tile_pool(name="singles", bufs=1) as singles, \
             tc.tile_pool(name="io", bufs=1) as io_pool, \
             tc.tile_pool(name="work", bufs=1) as work, \
             tc.tile_pool(name="ps", bufs=1, space="PSUM") as ps:

            whh_sb = singles.tile([128, KC, G], F16, tag="whh")
            nc.sync.dma_start(whh_sb, whh)
            whh8_sb = None
            if dr:
                whh8_sb = singles.tile([128, KC, G], F8, tag="whh8")
                nc.sync.dma_start(whh8_sb, whh8)
            wout_sb = singles.tile([128, KC, NTOK], F16, tag="wout")
            nc.sync.dma_start(wout_sb, wout)
            gi_sb = singles.tile([128, 2, 128], F16, tag="gi")
            nc.sync.dma_start(gi_sb, gi)
            bout_sb = singles.tile([1, 128], F16, tag="bout")
            nc.sync.dma_start(bout_sb, bout)
            ones_row = singles.tile([1, n_b], F16, tag="ones")
            nc.vector.memset(ones_row, 1.0)
            bhhn_sb = None
            if use_bhhn:
                bhhn_sb = singles.tile([1, LATENT], F16, tag="bhhn")
                nc.sync.dma_start(bhhn_sb, bhhn)

            class Chunk:
                pass

            def prologue(c):
                s = Chunk()
                s.cs = slice(c * n_b, (c + 1) * n_b)
                s.h0 = io_pool.tile([128, KC, n_b], F16, tag="h0", bufs=6, name="h0")
                nc.sync.dma_start(s.h0, latT[:, :, s.cs])
                s.oh = io_pool.tile([128, L, n_b], F16, tag="oh", bufs=6, name="oh")
                for g in range(4):
                    nc.sync.dma_start(s.oh[32 * g:32 * g + 8], ohd[:, :, s.cs])
                s.hist = io_pool.tile([128, L, KC, n_b], F16, tag="hist",
                                      bufs=min(5, max(2, n_chunks)), name="hist")
                if dr:
                    s.h8 = io_pool.tile([128, KC, n_b], F8, tag="h8", bufs=6,
                                        name="h8")
                    nc.gpsimd.tensor_copy(s.h8, s.h0)
                    s.h8s = {}
                return s

            def h_at(s, t, k):
                return s.h0[:, k, :] if t == 0 else s.hist[:, t - 1, k, :]

            def h8_at(s, t):
                return s.h8 if t == 0 else s.h8s[t - 1]

            def first_half(s, t):
                # n-part W_hh matmuls (emitted first: their PSUM slot frees
                # earliest and the r*gh_n product needs them before sigma(r))
                s.hn = ps.tile([128, 2, n_b], F32, tag="hn", bufs=2, name="hn")
                for m in range(2):
                    if dr >= 2:
                        nc.tensor.matmul(
                            s.hn[:, m, :],
                            lhsT=whh8_sb[:, :, 512 + 128 * m:640 + 128 * m],
                            rhs=h8_at(s, t), perf_mode=DR,
                            start=True, stop=not use_bhhn)
                    else:
                        for k in range(KC):
                            nc.tensor.matmul(
                                s.hn[:, m, :],
                                lhsT=whh_sb[:, k, 512 + 128 * m:640 + 128 * m],
                                rhs=h_at(s, t, k),
                                start=(k == 0),
                                stop=(k == KC - 1) and not use_bhhn)
                    if use_bhhn:
                        nc.tensor.matmul(
                            s.hn[:, m, :],
                            lhsT=bhhn_sb[:, 128 * m:128 * (m + 1)],
                            rhs=ones_row, start=False, stop=True)

                # rz gates: 4 onehot MMs (4-way row-tiled, start the psum) then
                # the W_hh MMs, accumulated per 128-row gate chunk.
                rz = [ps.tile([128, 2, n_b], F32, tag="rz", bufs=2, name=f"rz{j}")
                      for j in range(2)]
                for mc in range(4):
                    nc.tensor.matmul(
                        rz[mc // 2][:, mc % 2, :],
                        lhsT=gi_sb[32 * mc:32 * mc + 8, 0, :],
                        rhs=s.oh[32 * mc:32 * mc + 8, t, :],
                        start=True, stop=False, tile_position=(32 * mc, 0))
                for mc in range(4):
                    if dr >= 1:
                        nc.tensor.matmul(
                            rz[mc // 2][:, mc % 2, :],
                            lhsT=whh8_sb[:, :, 128 * mc:128 * (mc + 1)],
                            rhs=h8_at(s, t), perf_mode=DR,
                            start=False, stop=True)
                    else:
                        for k in range(KC):
                            nc.tensor.matmul(
                                rz[mc // 2][:, mc % 2, :],
                                lhsT=whh_sb[:, k, 128 * mc:128 * (mc + 1)],
                                rhs=h_at(s, t, k),
                                start=False, stop=(k == KC - 1))

                # sigmoids (r first: the p-mul below waits on it)
                s.sig = work.tile([128, 4, n_b], F16, tag="sig", bufs=4, name="sig")
                nc.scalar.activation(s.sig[:, 0:2, :], rz[0], AF.Sigmoid)
                nc.scalar.activation(s.sig[:, 2:4, :], rz[1], AF.Sigmoid)

                # p = r * gh_n, in place in PSUM (keeps has_written bits set)
                nc.vector.tensor_mul(s.hn, s.sig[:, 0:2, :], s.hn)

            def second_half(s, t):
                # accumulate gi_n on top of p (start=False: add where written)
                for m in range(2):
                    nc.tensor.matmul(
                        s.hn[:, m, :],
                        lhsT=gi_sb[32 * m:32 * m + 8, 1, :],
                        rhs=s.oh[32 * m:32 * m + 8, t, :],
                        start=False, stop=True, skip_group_check=True)
                nt = work.tile([128, 2, n_b], F16, tag="nt", bufs=2, name="nt")
                nc.scalar.activation(nt, s.hn, AF.Tanh)
                h_old = s.h0 if t == 0 else s.hist[:, t - 1, :, :]
                d = work.tile([128, 2, n_b], F16, tag="d", bufs=2, name="d")
                nc.vector.tensor_tensor(d, nt, h_old, ALU.subtract)
                e = work.tile([128, 2, n_b], F16, tag="e", bufs=2, name="e")
                (nc.gpsimd if eg else nc.vector).tensor_mul(e, s.sig[:, 2:4, :], d)
                nc.vector.tensor_tensor(s.hist[:, t, :, :], nt, e, ALU.subtract)
                if dr and t < L - 1:
                    h8n = io_pool.tile([128, KC, n_b], F8, tag="h8", bufs=6,
                                       name="h8n")
                    nc.gpsimd.tensor_copy(h8n, s.hist[:, t, :, :])
                    s.h8s[t] = h8n

            def logits_half(s, half):
                # steps 4*half..4*half+3 col-tiled into bank 0 of a psum tile
                # borrowed from alternating tags so neither rotation overloads
                tagn = "rz" if half == 0 else "hn"
                lgt = ps.tile([128, 2, n_b], F32, tag=tagn, bufs=2, name="lgt")
                nc.tensor.matmul(lgt[:, 0, :], lhsT=bout_sb, rhs=ones_row,
                                 start=True, stop=True)
                for g in range(4):
                    t = 4 * half + g
                    for k in range(KC):
                        nc.tensor.matmul(
                            lgt[32 * g:32 * g + 5, 0, :],
                            lhsT=wout_sb[:, k, :],
                            rhs=s.hist[:, t, k, :],
                            start=False, stop=(k == KC - 1),
                            tile_position=(0, 32 * g), skip_group_check=True)
                if half == 0:
                    s.lgsb = work.tile([128, 2, n_b], F16, tag="lg", bufs=2,
                                       name="lgsb")
                    nc.vector.tensor_copy(s.lgsb[:, 0, :], lgt[:, 0, :])
                else:
                    nc.scalar.copy(s.lgsb[:, 1, :], lgt[:, 0, :])
                for g in range(4):
                    t = 4 * half + g
                    nc.gpsimd.dma_start(out16[t, :, s.cs],
                                        s.lgsb[32 * g:32 * g + 5, half, :])

            # Flat skewed pipeline: chunk c runs step t at round 2c + t, so ~4
            # chunks are always in flight with no group barriers. Prologue DMAs
            # are emitted 3 rounds early; second halves lag one first-half so
            # no engine queue head-of-line blocks on the recurrence chain.
            stag = 2
            sts = {}
            fifo = []
            lag = 1 if n_chunks > 1 else 0
            n_rounds = stag * (n_chunks - 1) + L
            def pop_one():
                j, tj = fifo.pop(0)
                second_half(sts[j], tj)
                if tj == 3:
                    logits_half(sts[j], 0)
                elif tj == L - 1:
                    logits_half(sts[j], 1)
                return (j, tj)

            for r in range(n_rounds):
                for c in range(n_chunks):
                    if c not in sts and stag * c - 3 <= r:
                        sts[c] = prologue(c)
                for c in sorted(sts):
                    t = r - stag * c
                    if not (0 <= t < L):
                        continue
                    if (c, t - 1) in fifo:
                        while pop_one() != (c, t - 1):
                            pass
                    first_half(sts[c], t)
                    fifo.append((c, t))
                    if len(fifo) > lag:
                        pop_one()
            while fifo:
                pop_one()

    nc.compile()
    return nc


def make_in_maps(latent_context, target_sequence, emb_table, W_ih, W_hh,
                 b_ih, b_hh, W_out, b_out, b_core=B_CORE, mm="f16", dr=0):
    """Shard + lay out the inputs for each core. Layout-only host transforms."""
    lat = np.asarray(latent_context, dtype=np.float32)
    tok = np.asarray(target_sequence)
    emb = np.asarray(emb_table, dtype=np.float32)
    W_ih = np.asarray(W_ih, dtype=np.float32)
    W_hh = np.asarray(W_hh, dtype=np.float32)
    b_ih = np.asarray(b_ih, dtype=np.float32)
    b_hh = np.asarray(b_hh, dtype=np.float32)
    W_out = np.asarray(W_out, dtype=np.float32)
    b_out = np.asarray(b_out, dtype=np.float32)
    bout128 = np.zeros((1, 128), np.float32)
    for g in range(4):
        bout128[0, 32 * g:32 * g + NTOK] = b_out

    # gi table: gi_full[v, :] = emb[v] @ W_ih.T + b_ih (+ b_hh on the rz part)
    gi_full = emb @ W_ih.T + b_ih  # (V, G)
    gi_full[:, :512] += b_hh[:512]
    # pack: slot 0 row-group 32*mc <- rz chunk mc; slot 1 row-group 32*m <- n
    # chunk m. rows 6,7 of each group stay zero (K=8 onehot matmul).
    gi_pack = np.zeros((128, 2, 128), np.float32)
    for mc in range(4):
        gi_pack[32 * mc:32 * mc + V, 0, :] = gi_full[:, 128 * mc:128 * (mc + 1)]
    for m in range(2):
        gi_pack[32 * m:32 * m + V, 1, :] = gi_full[:, 512 + 128 * m:640 + 128 * m]

    # onehots for all 8 input tokens: tok_in = [START, tgt[:, :-1]]
    tok_in = np.concatenate(
        [np.full((tok.shape[0], 1), START, tok.dtype), tok[:, :L - 1]], axis=1)
    oh_all = (tok_in[None, :, :] == np.arange(8)[:, None, None])  # (8, B, L)
    oh_all = np.ascontiguousarray(
        np.transpose(oh_all, (0, 2, 1)).astype(np.float16))  # (8, L, B)

    latT = np.ascontiguousarray(
        lat.T.reshape(KC, 128, lat.shape[0]).transpose(1, 0, 2).astype(np.float16))
    whhT = np.ascontiguousarray(
        W_hh.T.reshape(KC, 128, G).transpose(1, 0, 2).astype(np.float16))
    import ml_dtypes
    whhT8 = np.ascontiguousarray(whhT.astype(ml_dtypes.float8_e4m3))
    woutT = np.ascontiguousarray(
        W_out.T.reshape(KC, 128, NTOK).transpose(1, 0, 2).astype(np.float16))
    gi_pack = gi_pack.astype(np.float16)

    use_bhhn = bool(np.any(b_hh[512:]))
    n_cores_eff = lat.shape[0] // b_core
    in_maps = []
    for i in range(n_cores_eff):
        sl = slice(i * b_core, (i + 1) * b_core)
        m = {
            "latT": np.ascontiguousarray(latT[:, :, sl]),
            "ohd": np.ascontiguousarray(oh_all[:, :, sl]),
            "gi": gi_pack,
            "whh": whhT,
            "wout": woutT,
            "bout": bout128.astype(np.float16),
        }
        if dr:
            m["whh8"] = whhT8
        if use_bhhn:
            m["bhhn"] = np.ascontiguousarray(
                b_hh[512:].reshape(1, LATENT).astype(np.float16))
        in_maps.append(m)
    return in_maps


_PROGRAM_CACHE = {}


DR_MODE = 0


EG_MODE = False


def _get_program(b_core, use_bhhn, dr, eg):
    key = (b_core, use_bhhn, dr, eg)
    if key not in _PROGRAM_CACHE:
        _PROGRAM_CACHE[key] = build_program(b_core=b_core, use_bhhn=use_bhhn,
                                            dr=dr, eg=eg)
    return _PROGRAM_CACHE[key]


def run(inputs, trace=False, b_core=B_CORE, mm="f16", dr=None, eg=None):
    if dr is None:
        dr = DR_MODE
    if eg is None:
        eg = EG_MODE
    in_maps = make_in_maps(b_core=b_core, dr=dr, **inputs)
    use_bhhn = "bhhn" in in_maps[0]
    nc = _get_program(b_core, use_bhhn, dr, eg)
    core_ids = list(range(len(in_maps)))
    res = bass_utils.run_bass_kernel_spmd(nc, in_maps, core_ids, trace=trace)
    outs = []
    for i in core_ids:
        o = res.results[i]["out16"]  # (L, NTOK, b_core) f16
        o = np.transpose(o, (2, 0, 1)).astype(np.float32)
        outs.append(o)
    return np.concatenate(outs, axis=0), res


def kernel(**inputs) -> np.ndarray:
    out, _ = run(inputs, trace=False)
    return out


# revision 15
# speedup vs baseline: 1.4809x; 1.4809x over previous
"""Trainium2 Bass kernel for nn_AutoregressiveRoutingHead (v2).

Model (per batch row b):
    tok_in = [START, tgt[0..6]]                       # teacher forcing, START=5
    x_t    = emb[tok_in[t]]                           # (HID,)
    gi     = x_t @ W_ih.T + b_ih                      # (768,)
    gh     = h @ W_hh.T + b_hh                        # (768,)
    r = sigmoid(gi_r + gh_r); z = sigmoid(gi_z + gh_z)
    n = tanh(gi_n + r * gh_n)
    h' = n - z*(n - h)
    logits_t = h' @ W_out.T + b_out                   # (5,)

Strategy (pure data parallel over batch, 65536 -> 8 x 8192):
  * All per-token quantities are host-precomputed: the latent h0 arrives
    pre-transposed/pre-cast as f16 [128, KC, b], the token onehots for all 8
    steps (incl. START at t=0) arrive as f16 [8, L, b], and the 6x768 gi table
    (emb @ W_ih.T + b_ih + b_hh_rz) is packed so each 128-row gate chunk's
    slice sits at partition rows 32*m (4-way row-tiled K=8 matmuls).
  * Steps are uniform: rz gates = onehot-MM (start) + 2 W_hh MMs per chunk
    into two 2-bank PSUM tiles, one sigmoid per tile; n-part = 4 W_hh MMs into
    a 2-bank PSUM tile, then DVE computes r*gh_n IN PLACE in PSUM and the PE
    accumulates gi_n on top (has_written bits survive the DVE overwrite), so
    tanh reads the finished preactivation straight from PSUM.
  * Logits are deferred: h'_t for all 8 steps is kept in SBUF; at chunk end 16
    col-tiled MMs put steps 4b+g at PSUM partitions 32g of bank b, one ACT
    copy extracts them, GPSIMD-queue DMAs write them out (f16; host upcasts).
  * 4 chunks in flight; per step-round the emission is software-pipelined
    (second half of the update lagged by one chunk) so no engine queue ever
    head-of-line blocks on the recurrence chain.
"""

import numpy as np

import concourse.bass as bass
import concourse.mybir as mybir
import concourse.tile as tile
from concourse import bacc, bass_utils

F32 = mybir.dt.float32
F16 = mybir.dt.float16
F8 = mybir.dt.float8e4
DR = mybir.MatmulPerfMode.DoubleRow
AF = mybir.ActivationFunctionType
ALU = mybir.AluOpType

N_CORES = 8
B = 65536
L = 8
LATENT = 256
HID = 128
NTOK = 5
V = NTOK + 1  # vocab incl <start>
START = NTOK
G = 3 * LATENT  # 768 gate rows
KC = LATENT // 128  # 2 contraction chunks

B_CORE = B // N_CORES


def build_program(b_core=B_CORE, n_b=512, group=4, use_bhhn=False, dr=0,
                  eg=False):
    """Build + compile the per-core Bass program (SPMD: same program, 8 cores).

    dr=0: all-f16 matmuls; dr=1: fp8 DoubleRow for the rz-gate W_hh matmuls;
    dr=2: fp8 DoubleRow for rz and n-gate W_hh matmuls.
    """
    nc = bacc.Bacc("TRN2", target_bir_lowering=False, debug=False)
    n_chunks = b_core // n_b
    assert n_chunks * n_b == b_core

    # ---- DRAM I/O ----------------------------------------------------------
    latT = nc.dram_tensor("latT", [128, KC, b_core], F16, kind="ExternalInput").ap()
    ohd = nc.dram_tensor("ohd", [8, L, b_core], F16, kind="ExternalInput").ap()
    gi = nc.dram_tensor("gi", [128, 2, 128], F16, kind="ExternalInput").ap()
    whh = nc.dram_tensor("whh", [128, KC, G], F16, kind="ExternalInput").ap()
    whh8 = None
    if dr:
        whh8 = nc.dram_tensor("whh8", [128, KC, G], F8, kind="ExternalInput").ap()
    wout = nc.dram_tensor("wout", [128, KC, NTOK], F16, kind="ExternalInput").ap()
    bout = nc.dram_tensor("bout", [1, 128], F16, kind="ExternalInput").ap()
    bhhn = None
    if use_bhhn:
        bhhn = nc.dram_tensor("bhhn", [1, LATENT], F16, kind="ExternalInput").ap()
    out16 = nc.dram_tensor("out16", [L, NTOK, b_core], F16, kind="ExternalOutput").ap()

    with tile.TileContext(nc) as tc:
        with tc.tile_pool(name="singles", bufs=1) as singles, \
             tc.tile_pool(name="io", bufs=1) as io_pool, \
             tc.tile_pool(name="work", bufs=1) as work, \
             tc.tile_pool(name="ps", bufs=1, space="PSUM") as ps:

            whh_sb = singles.tile([128, KC, G], F16, tag="whh")
            nc.sync.dma_start(whh_sb, whh)
            whh8_sb = None
            if dr:
                whh8_sb = singles.tile([128, KC, G], F8, tag="whh8")
                nc.sync.dma_start(whh8_sb, whh8)
            wout_sb = singles.tile([128, KC, NTOK], F16, tag="wout")
            nc.sync.dma_start(wout_sb, wout)
            gi_sb = singles.tile([128, 2, 128], F16, tag="gi")
            nc.sync.dma_start(gi_sb, gi)
            bout_sb = singles.tile([1, 128], F16, tag="bout")
            nc.sync.dma_start(bout_sb, bout)
            ones_row = singles.tile([1, n_b], F16, tag="ones")
            nc.vector.memset(ones_row, 1.0)
            bhhn_sb = None
            if use_bhhn:
                bhhn_sb = singles.tile([1, LATENT], F16, tag="bhhn")
                nc.sync.dma_start(bhhn_sb, bhhn)

            class Chunk:
                pass

            def prologue(c):
                s = Chunk()
                s.cs = slice(c * n_b, (c + 1) * n_b)
                s.h0 = io_pool.tile([128, KC, n_b], F16, tag="h0", bufs=6, name="h0")
                nc.sync.dma_start(s.h0, latT[:, :, s.cs])
                s.oh = io_pool.tile([128, L, n_b], F16, tag="oh", bufs=6, name="oh")
                for g in range(4):
                    nc.sync.dma_start(s.oh[32 * g:32 * g + 8], ohd[:, :, s.cs])
                s.hist = io_pool.tile([128, L, KC, n_b], F16, tag="hist",
                                      bufs=min(5, max(2, n_chunks)), name="hist")
                if dr:
                    s.h8 = io_pool.tile([128, KC, n_b], F8, tag="h8", bufs=6,
                                        name="h8")
                    nc.gpsimd.tensor_copy(s.h8, s.h0)
                    s.h8s = {}
                return s

            def h_at(s, t, k):
                return s.h0[:, k, :] if t == 0 else s.hist[:, t - 1, k, :]

            def h8_at(s, t):
                return s.h8 if t == 0 else s.h8s[t - 1]

            def first_half(s, t):
                # rz onehot MMs first: independent of h', give the PE queue
                # ready work and a 4-way packing opportunity.
                rz = [ps.tile([128, 2, n_b], F32, tag="rz", bufs=2, name=f"rz{j}")
                      for j in range(2)]
                for mc in range(4):
                    nc.tensor.matmul(
                        rz[mc // 2][:, mc % 2, :],
                        lhsT=gi_sb[32 * mc:32 * mc + 8, 0, :],
                        rhs=s.oh[32 * mc:32 * mc + 8, t, :],
                        start=True, stop=False, tile_position=(32 * mc, 0))

                s.hn = ps.tile([128, 2, n_b], F32, tag="hn", bufs=2, name="hn")
                for m in range(2):
                    if dr >= 2:
                        nc.tensor.matmul(
                            s.hn[:, m, :],
                            lhsT=whh8_sb[:, :, 512 + 128 * m:640 + 128 * m],
                            rhs=h8_at(s, t), perf_mode=DR,
                            start=True, stop=not use_bhhn)
                    else:
                        for k in range(KC):
                            nc.tensor.matmul(
                                s.hn[:, m, :],
                                lhsT=whh_sb[:, k, 512 + 128 * m:640 + 128 * m],
                                rhs=h_at(s, t, k),
                                start=(k == 0),
                                stop=(k == KC - 1) and not use_bhhn)
                    if use_bhhn:
                        nc.tensor.matmul(
                            s.hn[:, m, :],
                            lhsT=bhhn_sb[:, 128 * m:128 * (m + 1)],
                            rhs=ones_row, start=False, stop=True)

                # rz W_hh MMs, accumulated per 128-row gate chunk
                for mc in range(4):
                    if dr >= 1:
                        nc.tensor.matmul(
                            rz[mc // 2][:, mc % 2, :],
                            lhsT=whh8_sb[:, :, 128 * mc:128 * (mc + 1)],
                            rhs=h8_at(s, t), perf_mode=DR,
                            start=False, stop=True)
                    else:
                        for k in range(KC):
                            nc.tensor.matmul(
                                rz[mc // 2][:, mc % 2, :],
                                lhsT=whh_sb[:, k, 128 * mc:128 * (mc + 1)],
                                rhs=h_at(s, t, k),
                                start=False, stop=(k == KC - 1))

                # sigmoids (r first: the p-mul below waits on it)
                s.sig = work.tile([128, 4, n_b], F16, tag="sig", bufs=4, name="sig")
                nc.scalar.activation(s.sig[:, 0:2, :], rz[0], AF.Sigmoid)
                nc.scalar.activation(s.sig[:, 2:4, :], rz[1], AF.Sigmoid)

                # p = r * gh_n, in place in PSUM (keeps has_written bits set)
                nc.vector.tensor_mul(s.hn, s.sig[:, 0:2, :], s.hn)

            def second_half(s, t):
                # accumulate gi_n on top of p (start=False: add where written)
                for m in range(2):
                    row = 64 + 32 * m
                    nc.tensor.matmul(
                        s.hn[:, m, :],
                        lhsT=gi_sb[row:row + 8, 1, :],
                        rhs=s.oh[row:row + 8, t, :],
                        start=False, stop=True, skip_group_check=True,
                        tile_position=(row, 0))
                nt = work.tile([128, 2, n_b], F16, tag="nt", bufs=2, name="nt")
                nc.scalar.activation(nt, s.hn, AF.Tanh)
                h_old = s.h0 if t == 0 else s.hist[:, t - 1, :, :]
                d = work.tile([128, 2, n_b], F16, tag="d", bufs=2, name="d")
                nc.vector.tensor_tensor(d, nt, h_old, ALU.subtract)
                e = work.tile([128, 2, n_b], F16, tag="e", bufs=2, name="e")
                (nc.gpsimd if eg else nc.vector).tensor_mul(e, s.sig[:, 2:4, :], d)
                nc.vector.tensor_tensor(s.hist[:, t, :, :], nt, e, ALU.subtract)
                if dr and t < L - 1:
                    h8n = io_pool.tile([128, KC, n_b], F8, tag="h8", bufs=6,
                                       name="h8n")
                    nc.gpsimd.tensor_copy(h8n, s.hist[:, t, :, :])
                    s.h8s[t] = h8n

            def logits_half(s, half):
                # steps 4*half..4*half+3 col-tiled into bank 0 of a psum tile
                # borrowed from alternating tags so neither rotation overloads
                tagn = "rz" if half == 0 else "hn"
                lgt = ps.tile([128, 2, n_b], F32, tag=tagn, bufs=2, name="lgt")
                nc.tensor.matmul(lgt[:, 0, :], lhsT=bout_sb, rhs=ones_row,
                                 start=True, stop=True)
                for g in range(4):
                    t = 4 * half + g
                    for k in range(KC):
                        nc.tensor.matmul(
                            lgt[32 * g:32 * g + 5, 0, :],
                            lhsT=wout_sb[:, k, :],
                            rhs=s.hist[:, t, k, :],
                            start=False, stop=(k == KC - 1),
                            tile_position=(0, 32 * g), skip_group_check=True)
                if half == 0:
                    s.lgsb = work.tile([128, 2, n_b], F16, tag="lg", bufs=2,
                                       name="lgsb")
                    nc.vector.tensor_copy(s.lgsb[:, 0, :], lgt[:, 0, :])
                else:
                    nc.scalar.copy(s.lgsb[:, 1, :], lgt[:, 0, :])
                for g in range(4):
                    t = 4 * half + g
                    nc.gpsimd.dma_start(out16[t, :, s.cs],
                                        s.lgsb[32 * g:32 * g + 5, half, :])

            # Flat skewed pipeline: chunk c runs step t at round 2c + t, so ~4
            # chunks are always in flight with no group barriers. Prologue DMAs
            # are emitted 3 rounds early; second halves lag one first-half so
            # no engine queue head-of-line blocks on the recurrence chain.
            stag = 2
            sts = {}
            fifo = []
            lag = 1 if n_chunks > 1 else 0
            n_rounds = stag * (n_chunks - 1) + L
            def pop_one():
                j, tj = fifo.pop(0)
                second_half(sts[j], tj)
                if tj == 3:
                    logits_half(sts[j], 0)
                elif tj == L - 1:
                    logits_half(sts[j], 1)
                return (j, tj)

            for r in range(n_rounds):
                for c in range(n_chunks):
                    if c not in sts and stag * c - 3 <= r:
                        sts[c] = prologue(c)
                for c in sorted(sts):
                    t = r - stag * c
                    if not (0 <= t < L):
                        continue
                    if (c, t - 1) in fifo:
                        while pop_one() != (c, t - 1):
                            pass
                    first_half(sts[c], t)
                    fifo.append((c, t))
                    if len(fifo) > lag:
                        pop_one()
            while fifo:
                pop_one()

    nc.compile()
    return nc


def make_in_maps(latent_context, target_sequence, emb_table, W_ih, W_hh,
                 b_ih, b_hh, W_out, b_out, b_core=B_CORE, mm="f16", dr=0):
    """Shard + lay out the inputs for each core. Layout-only host transforms."""
    lat = np.asarray(latent_context, dtype=np.float32)
    tok = np.asarray(target_sequence)
    emb = np.asarray(emb_table, dtype=np.float32)
    W_ih = np.asarray(W_ih, dtype=np.float32)
    W_hh = np.asarray(W_hh, dtype=np.float32)
    b_ih = np.asarray(b_ih, dtype=np.float32)
    b_hh = np.asarray(b_hh, dtype=np.float32)
    W_out = np.asarray(W_out, dtype=np.float32)
    b_out = np.asarray(b_out, dtype=np.float32)
    bout128 = np.zeros((1, 128), np.float32)
    for g in range(4):
        bout128[0, 32 * g:32 * g + NTOK] = b_out

    # gi table: gi_full[v, :] = emb[v] @ W_ih.T + b_ih (+ b_hh on the rz part)
    gi_full = emb @ W_ih.T + b_ih  # (V, G)
    gi_full[:, :512] += b_hh[:512]
    # pack: slot 0 row-group 32*mc <- rz chunk mc; slot 1 row-group 32*m <- n
    # chunk m. rows 6,7 of each group stay zero (K=8 onehot matmul).
    gi_pack = np.zeros((128, 2, 128), np.float32)
    for mc in range(4):
        gi_pack[32 * mc:32 * mc + V, 0, :] = gi_full[:, 128 * mc:128 * (mc + 1)]
    for m in range(2):
        row = 64 + 32 * m
        gi_pack[row:row + V, 1, :] = gi_full[:, 512 + 128 * m:640 + 128 * m]

    # onehots for all 8 input tokens: tok_in = [START, tgt[:, :-1]]
    tok_in = np.concatenate(
        [np.full((tok.shape[0], 1), START, tok.dtype), tok[:, :L - 1]], axis=1)
    oh_all = (tok_in[None, :, :] == np.arange(8)[:, None, None])  # (8, B, L)
    oh_all = np.ascontiguousarray(
        np.transpose(oh_all, (0, 2, 1)).astype(np.float16))  # (8, L, B)

    latT = np.ascontiguousarray(
        lat.T.reshape(KC, 128, lat.shape[0]).transpose(1, 0, 2).astype(np.float16))
    whhT = np.ascontiguousarray(
        W_hh.T.reshape(KC, 128, G).transpose(1, 0, 2).astype(np.float16))
    import ml_dtypes
    whhT8 = np.ascontiguousarray(whhT.astype(ml_dtypes.float8_e4m3))
    woutT = np.ascontiguousarray(
        W_out.T.reshape(KC, 128, NTOK).transpose(1, 0, 2).astype(np.float16))
    gi_pack = gi_pack.astype(np.float16)

    use_bhhn = bool(np.any(b_hh[512:]))
    n_cores_eff = lat.shape[0] // b_core
    in_maps = []
    for i in range(n_cores_eff):
        sl = slice(i * b_core, (i + 1) * b_core)
        m = {
            "latT": np.ascontiguousarray(latT[:, :, sl]),
            "ohd": np.ascontiguousarray(oh_all[:, :, sl]),
            "gi": gi_pack,
            "whh": whhT,
            "wout": woutT,
            "bout": bout128.astype(np.float16),
        }
        if dr:
            m["whh8"] = whhT8
        if use_bhhn:
            m["bhhn"] = np.ascontiguousarray(
                b_hh[512:].reshape(1, LATENT).astype(np.float16))
        in_maps.append(m)
    return in_maps


_PROGRAM_CACHE = {}


DR_MODE = 0


EG_MODE = False


def _get_program(b_core, use_bhhn, dr, eg):
    key = (b_core, use_bhhn, dr, eg)
    if key not in _PROGRAM_CACHE:
        _PROGRAM_CACHE[key] = build_program(b_core=b_core, use_bhhn=use_bhhn,
                                            dr=dr, eg=eg)
    return _PROGRAM_CACHE[key]


def run(inputs, trace=False, b_core=B_CORE, mm="f16", dr=None, eg=None):
    if dr is None:
        dr = DR_MODE
    if eg is None:
        eg = EG_MODE
    in_maps = make_in_maps(b_core=b_core, dr=dr, **inputs)
    use_bhhn = "bhhn" in in_maps[0]
    nc = _get_program(b_core, use_bhhn, dr, eg)
    core_ids = list(range(len(in_maps)))
    res = bass_utils.run_bass_kernel_spmd(nc, in_maps, core_ids, trace=trace)
    outs = []
    for i in core_ids:
        o = res.results[i]["out16"]  # (L, NTOK, b_core) f16
        o = np.transpose(o, (2, 0, 1)).astype(np.float32)
        outs.append(o)
    return np.concatenate(outs, axis=0), res


def kernel(**inputs) -> np.ndarray:
    out, _ = run(inputs, trace=False)
    return out
